# revision 2
# baseline (speedup 1.0000x reference)
"""CrossAttentionHead TRN2 kernel (v2).

Full inputs -> full output. Shards batch (B=8) across 8 NeuronCores,
one batch element per core (pure data parallel, no collectives).

Per-core layout (xT staged host-side as bf16 [E=768, S=2048]):
  qT/kT/vT = W*.T @ xT + b*          ([H=128, S], weights stationary, bf16)
  vN       = blockwise transpose(vT)  ([S,H] natural, bf16)
  scores   sT[sk, sq] = kT_blk.T @ qT (psum f32, 1024-sq halves, ring of 2)
  es       = exp(sT * 1/sqrt(768))    (ScalarE, bf16 out)
  acc     += es                       (DVE bf16, for row sums)
  oT      += vN_blk.T @ es            (PV accumulate, [H, S] psum f32)
  rowsumT  = acc_blk.T @ ones         (PE, [sq,1] per 128-block -> [128,16])
  out      = transpose(oT) * (1/rowsumT)

Engine budget per core: PE ~ warm 3 + proj 15.5 + vN-T 2 + scores 13.7
+ AV 13.7 + finale ~4 us; ScalarE exp floor 4.19M elems ~ 32 us
(pipelined against PE's scores+AV); DVE drains/acc ~ 25 us.
Matmuls stream 1 cyc/col at 2.4 GHz regardless of bf16/f32r; bf16 is
used for halved DMA/SBUF traffic and 2x DVE rates. Softmax skips
max-subtraction: energy/sqrt(768) ~ N(0, 0.41^2), exp is in range.
Measured numerics vs fp32 reference: rel err ~5e-3 (gate 2e-2).
"""

import sys

if '/opt/trn_rl_repo' not in sys.path:
    sys.path.insert(0, '/opt/trn_rl_repo')

import numpy as np

B, S, E, H = 8, 2048, 768, 128
NCORES = 8
ST = S // 128           # 16 sk tiles
EC = E // 128           # 6 embed chunks
SCALE = float(1.0 / np.sqrt(np.float32(E)))

_CACHE = {}


def _build():
    from contextlib import ExitStack

    import concourse.bacc as bacc
    import concourse.mybir as mybir
    import concourse.tile as tile
    from concourse.masks import make_identity

    dt = mybir.dt
    f32 = dt.float32
    bf16 = dt.bfloat16
    AF = mybir.ActivationFunctionType

    nc = bacc.Bacc(None, target_bir_lowering=False)
    xT_d = nc.dram_tensor("xT", [E, S], dt.uint16, kind="ExternalInput")
    w_d = {}
    b_d = {}
    for nm in ("q", "k", "v"):
        w_d[nm] = nc.dram_tensor(f"W{nm}", [E, H], dt.uint16,
                                 kind="ExternalInput")
        b_d[nm] = nc.dram_tensor(f"b{nm}", [H], f32, kind="ExternalInput")
    out_d = nc.dram_tensor("out", [S, H], f32, kind="ExternalOutput")

    with tile.TileContext(nc) as tc:
        es_stack = ExitStack()
        with tc.tile_pool(name="data", bufs=1) as db, \
             tc.tile_pool(name="es", bufs=3) as esp, \
             tc.tile_pool(name="fin", bufs=4) as finp:
            identf = db.tile([128, 128], f32)
            make_identity(nc, identf[:])
            identb = db.tile([128, 128], bf16)
            nc.vector.tensor_copy(identb[:], identf[:])
            onesb = db.tile([128, 1], bf16)
            nc.vector.memset(onesb[:], 1.0)

            w_sb = {}
            b_sb = {}
            for nm in ("q", "k", "v"):
                w_sb[nm] = db.tile([128, EC, H], bf16, name=f"w_{nm}")
                nc.sync.dma_start(
                    out=w_sb[nm][:],
                    in_=w_d[nm].rearrange("(c p) d -> p c d", p=128)
                    .bitcast(bf16))
                b_sb[nm] = db.tile([128, 1], f32, name=f"b_{nm}")
                nc.sync.dma_start(out=b_sb[nm][:], in_=b_d[nm][:, None])
            xT = []
            for c in range(EC):
                t = db.tile([128, S], bf16, name=f"xT{c}")
                nc.sync.dma_start(
                    out=t[:], in_=xT_d[c * 128:(c + 1) * 128, :].bitcast(bf16))
                xT.append(t)

            qT = db.tile([128, S], bf16, name="qT")
            kT = db.tile([128, S], bf16, name="kT")
            vT = db.tile([128, S], bf16, name="vT")
            vN = db.tile([128, S], bf16, name="vN")
            acc = db.tile([128, S], bf16, name="acc")
            oT_sb = db.tile([128, S], f32, name="oT_sb")
            rcpT = db.tile([128, ST], f32, name="rcpT")

            # ---- projections (PE kept gap-free; warm ramps the clock) ----
            pq_cm = tc.tile_pool(name="pq", bufs=1, space="PSUM", side="left")
            pq = pq_cm.__enter__()
            q_ps = pq.tile([128, S], f32, tag="q")
            with tc.tile_pool(name="pw", bufs=1, space="PSUM",
                              side="left") as pw:
                wps = pw.tile([128, 128], f32, tag="w")
                for _ in range(6):
                    nc.tensor.matmul(wps[:], identf[:], identf[:],
                                     start=True, stop=True)
                for c in range(EC):
                    for n in range(4):
                        nc.tensor.matmul(
                            q_ps[:, n * 512:(n + 1) * 512],
                            w_sb["q"][:, c, :],
                            xT[c][:, n * 512:(n + 1) * 512],
                            start=(c == 0), stop=(c == EC - 1))
                    if c < EC - 1:
                        nc.tensor.matmul(wps[:], identf[:], identf[:],
                                         start=True, stop=True)
                        nc.tensor.matmul(wps[:], identf[:], identf[:],
                                         start=True, stop=True)
                wsink = db.tile([128, 128], f32, name="wsink")
                nc.vector.tensor_copy(wsink[:], wps[:])

            pk_cm = tc.tile_pool(name="pk", bufs=1, space="PSUM", side="right")
            pk = pk_cm.__enter__()
            k_ps = pk.tile([128, S], f32, tag="k")
            for c in range(EC):
                for n in range(4):
                    nc.tensor.matmul(
                        k_ps[:, n * 512:(n + 1) * 512],
                        w_sb["k"][:, c, :],
                        xT[c][:, n * 512:(n + 1) * 512],
                        start=(c == 0), stop=(c == EC - 1))
            # q drain (DVE) overlaps k/v matmuls
            nc.vector.tensor_scalar_add(qT[:], q_ps[:], b_sb["q"][:])
            pq_cm.__exit__(None, None, None)

            pv_cm = tc.tile_pool(name="pv", bufs=1, space="PSUM", side="left")
            pv = pv_cm.__enter__()
            v_ps = pv.tile([128, S], f32, tag="v")
            for c in range(EC):
                for n in range(4):
                    nc.tensor.matmul(
                        v_ps[:, n * 512:(n + 1) * 512],
                        w_sb["v"][:, c, :],
                        xT[c][:, n * 512:(n + 1) * 512],
                        start=(c == 0), stop=(c == EC - 1))
            # k drain split scalar/DVE
            nc.scalar.activation(kT[:, :1024], k_ps[:, :1024], AF.Identity,
                                 bias=b_sb["k"][:], scale=1.0)
            nc.vector.tensor_scalar_add(kT[:, 1024:], k_ps[:, 1024:],
                                        b_sb["k"][:])
            pk_cm.__exit__(None, None, None)

            # v drain in 4 slices alternating scalar/DVE
            for n in range(4):
                sl = slice(n * 512, (n + 1) * 512)
                if n % 2 == 0:
                    nc.scalar.activation(vT[:, sl], v_ps[:, sl], AF.Identity,
                                         bias=b_sb["v"][:], scale=1.0)
                else:
                    nc.vector.tensor_scalar_add(vT[:, sl], v_ps[:, sl],
                                                b_sb["v"][:])
            pv_cm.__exit__(None, None, None)

            # ---- main attention ----
            ps_cm = tc.tile_pool(name="ps", bufs=2, space="PSUM", side="left")
            psl = ps_cm.__enter__()

            es_t = {}

            def emit_scores(kt, h):
                t = psl.tile([128, 1024], f32, tag="s")
                for n in range(2):
                    q0 = h * 1024 + n * 512
                    nc.tensor.matmul(
                        t[:, n * 512:(n + 1) * 512],
                        kT[:, kt * 128:(kt + 1) * 128],
                        qT[:, q0:q0 + 512],
                        start=True, stop=True)
                return t

            s_half = [emit_scores(0, 0), emit_scores(0, 1)]

            # vN transposes (PE) fill the window while k/v drains finish;
            # psum ring on the right (pk's banks)
            with tc.tile_pool(name="pvnt", bufs=2, space="PSUM",
                              side="right") as pvnt:
                for j in range(ST):
                    pt = pvnt.tile([128, 128], bf16, tag="vt")
                    nc.tensor.transpose(
                        pt[:], vT[:, j * 128:(j + 1) * 128], identb[:])
                    nc.vector.tensor_copy(vN[:, j * 128:(j + 1) * 128], pt[:])

            poT_cm = tc.tile_pool(name="poT", bufs=1, space="PSUM",
                                  side="right")
            poT = poT_cm.__enter__()
            oT_ps = poT.tile([128, S], f32, tag="o")

            for kt in range(ST):
                es = esp.tile([128, S], bf16, tag="es")
                es_t[kt] = es
                for h in range(2):
                    nc.scalar.activation(
                        es[:, h * 1024:(h + 1) * 1024], s_half[h][:],
                        AF.Exp, scale=SCALE)
                if kt == 0:
                    nc.vector.tensor_copy(acc[:], es[:])
                else:
                    nc.vector.tensor_add(acc[:], acc[:], es[:])
                if kt < ST - 1:
                    s_half = [emit_scores(kt + 1, 0), emit_scores(kt + 1, 1)]
                for n in range(4):
                    nc.tensor.matmul(
                        oT_ps[:, n * 512:(n + 1) * 512],
                        vN[:, kt * 128:(kt + 1) * 128],
                        es[:, n * 512:(n + 1) * 512],
                        start=(kt == 0), stop=(kt == ST - 1))
            ps_cm.__exit__(None, None, None)

            # ---- finale ----
            pf_cm = tc.tile_pool(name="pf", bufs=1, space="PSUM", side="left")
            pf = pf_cm.__enter__()
            rsT_ps = pf.tile([128, ST], f32, tag="rs")
            for j in range(ST):
                nc.tensor.matmul(rsT_ps[:, j:j + 1],
                                 acc[:, j * 128:(j + 1) * 128],
                                 onesb[:], start=True, stop=True)
            nc.vector.reciprocal(rcpT[:], rsT_ps[:])

            # oT psum -> SBUF f32 (scalar + DVE halves)
            nc.scalar.activation(oT_sb[:, :1024], oT_ps[:, :1024],
                                 AF.Identity, scale=1.0)
            nc.vector.tensor_copy(oT_sb[:, 1024:], oT_ps[:, 1024:])
            poT_cm.__exit__(None, None, None)

            with tc.tile_pool(name="pft", bufs=3, space="PSUM",
                              side="left") as pft:
                for st in range(ST):
                    ft = pft.tile([128, 128], f32, tag="ft")
                    nc.tensor.transpose(
                        ft[:], oT_sb[:, st * 128:(st + 1) * 128], identf[:])
                    ob = finp.tile([128, 128], f32, tag="ob")
                    nc.vector.tensor_scalar_mul(ob[:], ft[:],
                                                rcpT[:, st:st + 1])
                    nc.sync.dma_start(
                        out=out_d[st * 128:(st + 1) * 128, :], in_=ob[:])
            pf_cm.__exit__(None, None, None)

    nc.finalize()
    return nc


def _get_nc():
    if "nc" not in _CACHE:
        _CACHE["nc"] = _build()
    return _CACHE["nc"]


def make_in_maps(x, Wq, bq, Wk, bk, Wv, bv):
    import ml_dtypes

    bf = ml_dtypes.bfloat16
    x = np.asarray(x, dtype=np.float32)
    shared = {
        "Wq": np.asarray(Wq, np.float32).astype(bf).view(np.uint16),
        "bq": np.asarray(bq, np.float32),
        "Wk": np.asarray(Wk, np.float32).astype(bf).view(np.uint16),
        "bk": np.asarray(bk, np.float32),
        "Wv": np.asarray(Wv, np.float32).astype(bf).view(np.uint16),
        "bv": np.asarray(bv, np.float32),
    }
    in_maps = []
    for b in range(NCORES):
        xTb = np.ascontiguousarray(x[b].T).astype(bf).view(np.uint16)
        in_maps.append({"xT": xTb, **shared})
    return in_maps


def kernel(x, enc_output, Wq, bq, Wk, bk, Wv, bv):
    from concourse.bass_utils import run_bass_kernel_spmd

    nc = _get_nc()
    in_maps = make_in_maps(x, Wq, bq, Wk, bk, Wv, bv)
    res = run_bass_kernel_spmd(nc, in_maps, list(range(NCORES)))
    out = np.stack([res.results[b]["out"] for b in range(NCORES)], axis=0)
    return out.astype(np.float32)
